# revision 54
# baseline (speedup 1.0000x reference)
"""Trainium2 Bass kernel for nn_DistanceProbe.

Computes, for batch [B=8, S=2048, H=768] and proj [H=768, R=768]:
    t  = batch @ proj                                # [B, S, R]
    d2 = ||t_i||^2 + ||t_j||^2 - 2 t_i . t_j         # [B, S, S]

Sharding: data-parallel over B across the 8 NeuronCores (one batch
element per core). Host pre-transposes each batch slice (xT = batch[b].T)
and quantizes xT / proj to fp8e4 (e4m3); the device returns d2 in bf16
and the host upcasts to f32. Both roundings are far inside the 2e-2
relative-error budget and cut HBM traffic ~2.4x while enabling the PE
array's fp8 DoubleRow mode (K=256 per matmul at 0.5 cycles/row = 4x the
fp32r MAC rate).

Per-core device algorithm (PE-bound; every matmul is fp8 DoubleRow):
  1. Phase B: tT[r, s] = sum_h (x8 + rx)[h, s] * p8[h, r] as K=1536
     fp8-DR contractions (6 instructions per 512-col group, two groups
     per PSUM tile); ACT quantizes PSUM back to fp8 (t8).
  2. sq[i] = sum_r t8[r, i]^2 extracted from the diagonals of 16
     prepass dots blocks (4 per PSUM tile): mask by a tiled identity
     (DVE), free-dim reduce (DVE -> sq bias column), partition all-reduce
     (Pool); DVE then builds vq = fp8 pair [-sq/4; residual] used below.
     d2(i,i) == 0 exactly up to the vq split (~1e-3 relative).
  3. dots upper triangle: per 128-row strip, 3 fp8-DR matmuls plus a
     4th fp8-DR "fold" matmul per group (stationary = constant 2, moving
     = vq, K padded to 32 partitions) so PSUM = dots - sq_j/2. A single
     epilogue pass (ACT identity / DVE tensor_scalar, rotation p1dve)
     computes -2*PSUM + sq_i = d2 straight into the bf16 upper store.
     relu is intentionally omitted: negatives can only be quantization
     noise already counted in the error budget.
  4. lower triangle: PE-transposes of the stored upper bf16 blocks
     (8 per PSUM tile), copied to a row buffer by DVE/ACT; full rows
     stream out in at most 2 large DMAs per 128-row strip, mirrors
     trailing the dots by one row so the PE queue never blocks.

Scheduling: phase D is split into a [jstart, 1024) pass over rows 0-7
and a [1024, 2048) pass over all rows so early segments only depend on
the first half of t8; the next rep's input DMAs are emitted mid-rep
(ahead of this rep's output DMAs in the sync queue) and all input/t8
tiles are double-buffered, letting consecutive reps overlap.

`reps` repeats the whole body inside one NEFF (used by test.py to
measure steady-state HW time by differencing two rep counts).
"""

import numpy as np

import concourse.bass as bass
import concourse.tile as tile
from concourse import bacc
from concourse import masks
from concourse import mybir
from concourse.bass_utils import run_bass_kernel_spmd

B, S, H, R = 8, 2048, 768, 768
N_CORES = 8
P = 128
KT = H // P      # 6 k-tiles over H (and over R: H == R)
IT = S // P      # 16 row tiles
NC_ = 512        # matmul group width (one PSUM bank of fp32)
SEG = 1024       # epilogue segment width (two PSUM banks)

F32 = mybir.dt.float32
BF16 = mybir.dt.bfloat16
F8 = mybir.dt.float8e4
DR = mybir.MatmulPerfMode.DoubleRow
ALU = mybir.AluOpType
AFT = mybir.ActivationFunctionType

# st2s (upper-triangle store) row offsets: row it holds cols [128*it, S)
ROW_OFF = []
_off = 0
for _it in range(IT):
    ROW_OFF.append(_off)
    _off += S - P * _it
ST2_COLS = _off  # 17408


def _row_groups(it):
    """Matmul groups for row-strip it: [j0, j1) spans aligned to NC_."""
    j = it * P
    out = []
    while j < S:
        j1 = min((j // NC_ + 1) * NC_, S)
        out.append((j, j1))
        j = j1
    return out


def build_nc(reps=1, ablate=frozenset(), p1dve=4, mirdve=2, dstgb=3, mstgb=3, mdelay=1, p1hi=2, pbigb=3, pmb=2, pqsep=0, qdve=0):
    nc = bacc.Bacc("TRN2", target_bir_lowering=False, debug=False,
                   num_devices=N_CORES)

    # x8c holds [fp8(x); fp8(x - fp8(x))] stacked along H: the two-term fp8
    # split keeps the phase-B product accurate to ~0.06% on the x side
    x8_d = nc.dram_tensor("x8c", [2 * H, S], F8, kind="ExternalInput")
    p8_d = nc.dram_tensor("p8", [H, R], F8, kind="ExternalInput")
    out_d = nc.dram_tensor("out", [S, S], BF16, kind="ExternalOutput")

    x8_r = x8_d.rearrange("(kt p) s -> p kt s", p=P)
    p8_r = p8_d.rearrange("(kt p) r -> p kt r", p=P)

    with tile.TileContext(nc) as tc:
        with tc.tile_pool(name="persist", bufs=1) as sb, \
             tc.tile_pool(name="io", bufs=2) as io, \
             tc.tile_pool(name="mstage", bufs=mstgb) as mstg, \
             tc.tile_pool(name="dstage", bufs=dstgb) as dstg, \
             tc.tile_pool(name="pbig", bufs=pbigb, space="PSUM") as pbig, \
             tc.tile_pool(name="pm", bufs=pmb, space="PSUM") as pmp, \
             tc.tile_pool(name="pq", bufs=max(pqsep, 1), space="PSUM") as pqp:

            st2s = sb.tile([P, ST2_COLS], BF16, name="st2s", tag="st2s")
            sqcol = sb.tile([P, IT], F32, name="sqcol", tag="sqcol")
            identf4 = sb.tile([P, 4 * P], F32, name="identf4", tag="id4")
            identb = sb.tile([P, P], BF16, name="identb", tag="identb")
            # vq row 0 holds the fp8 pair [-sq/4; residual]; lhs2 row 0 is
            # the constant 2.0. Rows 1-31 stay zero so the DoubleRow fold
            # matmul (K padded to 32 partitions) adds exactly -sq_j/2 to
            # every dots accumulation group.
            vq = sb.tile([32, 2, S], F8, name="vq", tag="vq")
            lhs2 = sb.tile([32, 2, P], F8, name="lhs2", tag="lhs2")

            nc.vector.memset(identf4[:], 0.0)
            for k in range(4):
                masks.make_identity(nc, identf4[:, k * P:(k + 1) * P],
                                    nomemset=True)
            nc.vector.tensor_copy(identb[:], identf4[:, 0:P])
            nc.vector.memset(vq[:], 0.0)
            nc.vector.memset(lhs2[:], 0.0)
            nc.vector.memset(lhs2[0:1, :, :], 2.0)

            def emit_loads():
                # inputs and t8 are double-buffered (io pool, bufs=2); the
                # next rep's loads are emitted mid-rep (before this rep's
                # output DMAs enter the in-order sync queue) so the input
                # transfers overlap this rep's phase D
                x8 = io.tile([P, 2 * KT, S], F8, name="x8", tag="x8")
                p8 = io.tile([P, KT, R], F8, name="p8", tag="p8")
                t8 = io.tile([P, KT, S], F8, name="t8", tag="t8")
                nc.sync.dma_start(p8[:, :, :], p8_r[:, :, :])
                for sc in range(4):
                    nc.sync.dma_start(x8[:, :, sc * NC_:(sc + 1) * NC_],
                                      x8_r[:, :, sc * NC_:(sc + 1) * NC_])
                return x8, p8, t8

            def emit_body(tiles, preload):
                x8, p8, t8 = tiles

                # ---- phase B: tT = projT @ x, quantize to fp8 ----
                # diag/sq extraction for chunk sc is emitted after chunk
                # sc+1's matmuls: the PE queue is in-order, so this keeps
                # the diag matmuls (which wait on chunk sc's quantize) from
                # head-of-line-blocking the next chunk's phase-B matmuls
                def emit_diag(sc):
                    pool = pqp if pqsep else pmp
                    pq4 = pool.tile([P, 4 * P], F32, name="pq4",
                                    tag="pq" if pqsep else "mp")
                    for itl in range(4):
                        it = sc * 4 + itl
                        for k2 in range(3):
                            nc.tensor.matmul(
                                pq4[:, itl * P:(itl + 1) * P],
                                t8[:, 2 * k2:2 * k2 + 2, it * P:(it + 1) * P],
                                t8[:, 2 * k2:2 * k2 + 2, it * P:(it + 1) * P],
                                start=(k2 == 0), stop=(k2 == 2),
                                perf_mode=DR)
                    dg = dstg.tile([P, 4 * P], F32, name="dg", tag="dg")
                    nc.vector.tensor_tensor(dg[:], pq4[:], identf4[:],
                                            ALU.mult)
                    nc.vector.tensor_reduce(
                        sqcol[:, 4 * sc:4 * sc + 4],
                        dg[:].rearrange("p (a b) -> p a b", a=4),
                        mybir.AxisListType.X, ALU.add)
                    ch = slice(sc * NC_, (sc + 1) * NC_)
                    ar = dstg.tile([P, NC_], F32, name="ar", tag="dg")
                    nc.gpsimd.partition_all_reduce(
                        ar[:], dg[:], P, bass.bass_isa.ReduceOp.add)
                    # fp8 hi/lo pair for the fold row: hi = fp8(-sq/4),
                    # lo = fp8(-sq/4 - hi); on DVE because this chain gates
                    # the phase-D fold matmuls
                    sqr = ar[0:1, :]
                    l32 = dstg.tile([1, NC_], F32, name="l32", tag="l32")
                    nc.vector.tensor_scalar_mul(vq[0:1, 0, ch], sqr, -0.25)
                    nc.vector.scalar_tensor_tensor(
                        l32[:], sqr, -0.25, vq[0:1, 0, ch],
                        ALU.mult, ALU.subtract)
                    nc.vector.tensor_copy(vq[0:1, 1, ch], l32[:])

                qi = 0
                for sc in range(4):
                    for rtp in range(KT // 2):
                        pt = pbig.tile([P, SEG], F32, name="pt", tag="pb")
                        for half in range(2):
                            rt = 2 * rtp + half
                            for k2 in range(6):
                                pk = (2 * k2) % KT
                                nc.tensor.matmul(
                                    pt[:, half * NC_:(half + 1) * NC_],
                                    p8[:, pk:pk + 2,
                                       rt * P:(rt + 1) * P],
                                    x8[:, 2 * k2:2 * k2 + 2,
                                       sc * NC_:(sc + 1) * NC_],
                                    start=(k2 == 0), stop=(k2 == 5),
                                    perf_mode=DR)
                        dst = t8[:, 2 * rtp:2 * rtp + 2,
                                 sc * NC_:(sc + 1) * NC_]
                        src = pt[:].rearrange("p (a b) -> p a b", a=2)
                        if 'quant' in ablate:
                            dst = t8[:, 2 * rtp:2 * rtp + 2,
                                     sc * NC_:sc * NC_ + 1]
                            src = pt[:, 0:2].rearrange("p (a b) -> p a b", a=2)
                        if qdve and qi % qdve == qdve - 1:
                            nc.vector.tensor_copy(dst, src)
                        else:
                            nc.scalar.copy(dst, src)
                        qi += 1

                    if sc >= 1 and 'sq' not in ablate:
                        emit_diag(sc - 1)

                # next rep's input DMAs enter the sync queue here, ahead
                # of this rep's output DMAs
                nxt = preload()

                # ---- phase D: upper-triangle dots + epilogue + mirrors ----
                si = 0

                def emit_row_dots(it, jlo, jhi):
                    nonlocal si
                    jstart = it * P
                    groups = [g for g in _row_groups(it)
                              if g[0] >= jlo and g[1] <= jhi]
                    for s0 in range(0, len(groups), 2):
                        gs = groups[s0:s0 + 2]
                        j0 = gs[0][0]
                        j1 = gs[-1][1]
                        pd = pbig.tile([P, SEG], F32, name="pd", tag="pb")
                        off0 = NC_ - (gs[0][1] - gs[0][0])
                        for gi, (ja, jb) in enumerate(gs):
                            o = off0 if gi == 0 else NC_
                            for k2 in range(3):
                                nc.tensor.matmul(
                                    pd[:, o:o + (jb - ja)],
                                    t8[:, 2 * k2:2 * k2 + 2,
                                       it * P:(it + 1) * P],
                                    t8[:, 2 * k2:2 * k2 + 2, ja:jb],
                                    start=(k2 == 0), stop=False,
                                    perf_mode=DR)
                            # fold row: psum += 2 * (-sq_j/4 hi+lo pair)
                            nc.tensor.matmul(
                                pd[:, o:o + (jb - ja)],
                                lhs2[:, :, :],
                                vq[:, :, ja:jb],
                                start=False, stop=True,
                                perf_mode=DR)
                        w = j1 - j0
                        if 'epi' in ablate:
                            w = 1
                            j1 = j0 + 1
                        # single epilogue pass: st2 = -2*(dots - sq_j/2)
                        # + sq_i = d2, straight from PSUM to bf16
                        do = ROW_OFF[it] + (j0 - jstart)
                        if si % p1dve == p1dve - 1:
                            nc.vector.tensor_scalar(
                                st2s[:, do:do + w], pd[:, off0:off0 + w],
                                -2.0, sqcol[:, it:it + 1],
                                ALU.mult, ALU.add)
                        else:
                            nc.scalar.activation(
                                st2s[:, do:do + w], pd[:, off0:off0 + w],
                                AFT.Identity,
                                bias=sqcol[:, it:it + 1], scale=-2.0)
                        si += 1

                # mirror of row it: transpose stored blocks (it', it), DMA.
                # Emitted one row late (after row it+1's dots) so the PE
                # queue never head-of-line-blocks on row it's last pass2.
                def emit_row_mirror(it):
                    jstart = it * P
                    mb = mstg.tile([P, (IT - 1) * P], BF16, name="mb",
                                   tag="mb")
                    for g0 in range(0, it, 8):
                        g1 = min(g0 + 8, it)
                        mp = pmp.tile([P, 8 * P], BF16, name="mp", tag="mp")
                        for k, itp in enumerate(range(g0, g1)):
                            src = ROW_OFF[itp] + (it - itp) * P
                            nc.tensor.transpose(
                                mp[:, k * P:(k + 1) * P],
                                st2s[:, src:src + P],
                                identb[:])
                        cdst = mb[:, g0 * P:g1 * P]
                        csrc = mp[:, 0:(g1 - g0) * P]
                        if it % mirdve != 0:
                            nc.vector.tensor_copy(cdst, csrc)
                        else:
                            nc.scalar.copy(cdst, csrc)
                    if 'odma' not in ablate:
                        nc.sync.dma_start(
                            out_d[jstart:jstart + P, 0:jstart],
                            mb[:, 0:jstart])

                # pass 1: segments inside cols [jstart, 1024) — these only
                # need the first two t8 column chunks, so they overlap the
                # back half of phase B
                for it in range(8 if p1hi == 2 else 12):
                    emit_row_dots(it, 0, p1hi * NC_)
                    if it == 1 and 'sq' not in ablate:
                        emit_diag(3)
                # pass 2: segments in cols [1024, 2048), all rows; upper
                # DMA once a row completes; mirrors trail by one row
                for it in range(IT):
                    emit_row_dots(it, p1hi * NC_, S)
                    if 'odma' not in ablate:
                        nc.sync.dma_start(
                            out_d[it * P:(it + 1) * P, it * P:S],
                            st2s[:, ROW_OFF[it]:ROW_OFF[it] + (S - it * P)])
                    if it >= mdelay + 1 and 'mirror' not in ablate:
                        emit_row_mirror(it - mdelay)
                if 'mirror' not in ablate:
                    for it in range(IT - mdelay, IT):
                        emit_row_mirror(it)
                return nxt

            cur = emit_loads()
            for r in range(reps):
                preload = emit_loads if r + 1 < reps else (lambda: None)
                cur = emit_body(cur, preload)

    nc.finalize()
    return nc


_NC_CACHE = {}


def get_nc(reps=1):
    if reps not in _NC_CACHE:
        _NC_CACHE[reps] = build_nc(reps)
    return _NC_CACHE[reps]


def make_in_maps(batch, proj):
    f8 = mybir.dt.np(F8)
    p8 = np.ascontiguousarray(proj).astype(f8)
    maps = []
    for b in range(B):
        xT = np.ascontiguousarray(batch[b].T).astype(np.float32)
        x8 = xT.astype(f8)
        rx = (xT - x8.astype(np.float32)).astype(f8)
        maps.append({"x8c": np.concatenate([x8, rx], axis=0), "p8": p8})
    return maps


def kernel(batch, proj):
    assert batch.shape == (B, S, H) and proj.shape == (H, R)
    nc = get_nc()
    in_maps = make_in_maps(batch, proj)
    res = run_bass_kernel_spmd(nc, in_maps, core_ids=list(range(N_CORES)))
    out = np.stack([np.asarray(res.results[b]["out"]) for b in range(B)],
                   axis=0)
    return out.astype(np.float32)
